# revision 29
# baseline (speedup 1.0000x reference)
# Trainium2 Bass kernel for a ViT-style transformer block.
#   x = x + proj(attn(LN1(x)));  x = x + fc2(gelu(fc1(LN2(x))))
# B=32, N=577, C=1024, H=16, D=64, HID=4096.
#
# Distribution: pure data-parallel over batch, 4 images per NeuronCore.
# Per-core token layout: each image padded 577 -> 640 tokens, so a core
# processes T = 4*640 = 2560 tokens = 20 tiles of 128.
#
# All matmuls run in bf16 (fp32 PSUM accumulation); LN stats, softmax
# denominators and the residual stream stay fp32.
#
# Attention is computed keys-major: S^T = K_T^T @ Q_T per (b,h), exp on
# ScalarE (no max subtraction -- logits are O(1) here), AV as
# out^T = V_ext^T @ P^T where V_ext carries an extra ones column whose
# output row is the softmax denominator; normalization multiplies by a
# reciprocal row replicated across partitions on GPSIMD.  Activations
# cross phases feature-major (features on partitions) so matmul
# contractions never need activation transposes except the two LN
# outputs (PE transpose).  Q/K projection chunks are computed on demand
# inside the attention loop (2 head's worth at a time) which overlaps
# projection matmuls with attention and keeps SBUF small.

import numpy as np
import ml_dtypes

import concourse.bass as bass
import concourse.mybir as mybir
import concourse.tile as tile
from concourse import bacc
from concourse import bass_utils
from concourse.masks import make_identity

B, N, C = 32, 577, 1024
H, D = 16, 64
HID = 4 * C
EPS = 1e-5
SCALE = D ** -0.5

NCORES = 8
BPC = B // NCORES          # batches per core
NP = 640                   # padded tokens per batch (5 * 128)
T = BPC * NP               # 2560 padded tokens per core
TT = T // 128              # 20 token tiles
CC = C // 128              # 8 feature chunks
MT = NP // 128             # 5 key tiles per batch
HC = HID // 128            # 32 hidden chunks
CH = 512                   # MLP token chunk
NU = T // CH               # 10 MLP chunks

FP32 = mybir.dt.float32
BF16 = mybir.dt.bfloat16
AF = mybir.ActivationFunctionType
ALU = mybir.AluOpType
BF16NP = ml_dtypes.bfloat16


def _declare_io(nc):
    xp = nc.dram_tensor("xp", [T, C], FP32, kind="ExternalInput")
    wqkT = nc.dram_tensor("wqkT", [C, 2 * C], BF16, kind="ExternalInput")
    wvT = nc.dram_tensor("wvT", [C, C], BF16, kind="ExternalInput")
    wpT = nc.dram_tensor("wpT", [C, C], BF16, kind="ExternalInput")
    wf1T = nc.dram_tensor("wf1T", [C, HID], BF16, kind="ExternalInput")
    wf2T = nc.dram_tensor("wf2T", [HID, C], BF16, kind="ExternalInput")
    g1 = nc.dram_tensor("g1", [C], FP32, kind="ExternalInput")
    b1 = nc.dram_tensor("b1", [C], FP32, kind="ExternalInput")
    g2 = nc.dram_tensor("g2", [C], FP32, kind="ExternalInput")
    b2 = nc.dram_tensor("b2", [C], FP32, kind="ExternalInput")
    bpj = nc.dram_tensor("bpj", [C], FP32, kind="ExternalInput")
    bf1 = nc.dram_tensor("bf1", [HID], FP32, kind="ExternalInput")
    bf2 = nc.dram_tensor("bf2", [C], FP32, kind="ExternalInput")
    out = nc.dram_tensor("out", [T, C], FP32, kind="ExternalOutput")
    # DRAM scratch
    x2d = nc.dram_tensor("x2d", [T, C], FP32, kind="Internal")
    y2Td = nc.dram_tensor("y2Td", [C, T], BF16, kind="Internal")
    return (xp, wqkT, wvT, wpT, wf1T, wf2T, g1, b1, g2, b2, bpj, bf1, bf2,
            out, x2d, y2Td)


def _build_once(nc, tc, io):
    (xp, wqkT, wvT, wpT, wf1T, wf2T, g1, b1, g2, b2, bpj, bf1, bf2,
     out, x2d, y2Td) = io
    xp_r = xp[:].rearrange("(t p) c -> t p c", p=128)
    out_r = out[:].rearrange("(t p) c -> t p c", p=128)
    x2d_r = x2d[:].rearrange("(t p) c -> t p c", p=128)
    y2Td_r = y2Td[:].rearrange("(o p) t -> p o t", p=128)
    wqkT_r = wqkT[:].rearrange("(o p) m -> p o m", p=128)

    def bcast_row(ap1d, parts=128):
        # DRAM [C] -> [parts, C] partition-broadcast AP
        return bass.AP(tensor=ap1d.tensor, offset=ap1d.offset,
                       ap=[[0, parts]] + list(ap1d.ap))

    with tc.tile_pool(name="const", bufs=1) as const, \
         tc.tile_pool(name="aff", bufs=1) as aff:
        eps_t = const.tile([128, 1], FP32)
        nc.vector.memset(eps_t, EPS)
        ident = const.tile([128, 128], FP32)
        make_identity(nc, ident)
        bf1_s = const.tile([128, HC], FP32)
        nc.scalar.dma_start(bf1_s, bf1[:].rearrange("(o p) -> p o", p=128))
        # LN gains/biases feature-major [128, CC]: applied per-partition at
        # the transpose eviction (features sit on partitions there)
        g1c = const.tile([128, CC], FP32)
        nc.scalar.dma_start(g1c, g1[:].rearrange("(o p) -> p o", p=128))
        b1c = const.tile([128, CC], FP32)
        nc.scalar.dma_start(b1c, b1[:].rearrange("(o p) -> p o", p=128))
        g2c = const.tile([128, CC], FP32)
        nc.scalar.dma_start(g2c, g2[:].rearrange("(o p) -> p o", p=128))
        b2c = const.tile([128, CC], FP32)
        nc.scalar.dma_start(b2c, b2[:].rearrange("(o p) -> p o", p=128))
        # staging area for the first quarter of fc1 weights: loaded during
        # phase 5 so the MLP's first matmuls start with zero DMA wait
        wf1a = const.tile([128, CC, 1024], BF16)
        wf1T_r = wf1T[:].rearrange("(o p) m -> p o m", p=128)

        def load_row(name, src):
            t_ = aff.tile([128, C], FP32, tag=name, name=name + "_row")
            nc.sync.dma_start(t_, bcast_row(src[:]))
            return t_

        def layernorm(x_t, pool_tmp):
            """x_t [128,C] fp32 -> returns [128,C] fp32 = (x - mu) * rstd"""
            st = pool_tmp.tile([128, 2, 6], FP32, tag="bnst", name="st")
            nc.vector.bn_stats(st[:, 0], x_t[:, 0:512])
            nc.vector.bn_stats(st[:, 1], x_t[:, 512:1024])
            mv = pool_tmp.tile([128, 2], FP32, tag="bnmv", name="mv")
            nc.vector.bn_aggr(mv, st)
            rstd = pool_tmp.tile([128, 1], FP32, tag="rstd", name="rstd")
            nc.scalar.activation(rstd, mv[:, 1:2], AF.Sqrt, bias=eps_t)
            nc.vector.reciprocal(rstd, rstd)
            t0 = pool_tmp.tile([128, C], FP32, tag="t0", name="t0")
            nc.vector.tensor_scalar(t0, x_t, scalar1=mv[:, 0:1], scalar2=rstd,
                                    op0=ALU.subtract, op1=ALU.mult)
            return t0

        def transpose_tile(y_t, dst_ap, pool_ps, tag, gc, bc):
            """y_t [128, C] fp32 -> dst_ap [128, CC, 128] bf16 feature-major
            strip; the eviction applies the per-feature LN gain/bias, which
            are per-PARTITION scalars after the transpose."""
            pt = pool_ps.tile([128, CC, 128], FP32, tag=tag, name=tag)
            for cc in range(CC):
                nc.tensor.transpose(pt[:, cc, :], y_t[:, cc * 128:(cc + 1) * 128],
                                    ident)
            for cc in range(CC):
                nc.scalar.activation(dst_ap[:, cc], pt[:, cc], AF.Identity,
                                     bias=bc[:, cc:cc + 1], scale=gc[:, cc:cc + 1])

        with tc.tile_pool(name="wp", bufs=1) as p_wp, \
             tc.tile_pool(name="attnT", bufs=1) as p_aT, \
             tc.tile_pool(name="v", bufs=1) as p_v, \
             tc.tile_pool(name="y1T", bufs=1) as p_y1T:
            attnT = p_aT.tile([128, CC, T], BF16)
            v_s = p_v.tile([128, TT, H, 65], BF16)
            nc.vector.memset(v_s[:, :, :, 64:65], 1.0)
            y1T = p_y1T.tile([128, CC, T], BF16)

            # ===== Phase 1+2 fused: LN1 + transpose + V projection =====
            with tc.tile_pool(name="wv", bufs=1) as p_wv, \
                 tc.tile_pool(name="s1", bufs=3) as s1, \
                 tc.tile_pool(name="s1p", bufs=2, space="PSUM") as s1p, \
                 tc.tile_pool(name="s2p", bufs=4, space="PSUM") as s2p:
                wvT_s = p_wv.tile([128, CC, C], BF16)
                nc.scalar.dma_start(wvT_s, wvT[:].rearrange("(o p) m -> p o m", p=128))
                for t in range(TT):
                    x_t = s1.tile([128, C], FP32, tag="x", name="x_t")
                    nc.sync.dma_start(x_t, xp_r[t])
                    y1 = layernorm(x_t, s1)
                    transpose_tile(y1, y1T[:, :, t * 128:(t + 1) * 128], s1p,
                                   "pst1", g1c, b1c)
                    for n2 in range(2):
                        ps = s2p.tile([128, 512], FP32, tag="ps_v", name="ps_v")
                        for cc in range(CC):
                            nc.tensor.matmul(ps,
                                             lhsT=y1T[:, cc, t * 128:(t + 1) * 128],
                                             rhs=wvT_s[:, cc, n2 * 512:(n2 + 1) * 512],
                                             start=(cc == 0), stop=(cc == CC - 1))
                        nc.scalar.copy(v_s[:, t, n2 * 8:(n2 + 1) * 8, 0:64],
                                       ps.rearrange("p (h d) -> p h d", h=8))

            # ===== Phase 3+4 fused: per head-pair QK projection + attention ===
            # prefetch proj weights (needed from phase 5)
            wpT_s = p_wp.tile([128, CC, C], BF16)
            nc.sync.dma_start(wpT_s, wpT[:].rearrange("(o p) m -> p o m", p=128))
            with tc.tile_pool(name="qk", bufs=2) as p_qk, \
                 tc.tile_pool(name="wqk", bufs=2) as p_wqk, \
                 tc.tile_pool(name="sexp", bufs=2) as p_se, \
                 tc.tile_pool(name="srow", bufs=2) as p_sr, \
                 tc.tile_pool(name="ps_qk", bufs=2, space="PSUM") as pp_qk, \
                 tc.tile_pool(name="ps_s", bufs=2, space="PSUM") as pp_s, \
                 tc.tile_pool(name="ps_o", bufs=1, space="PSUM") as pp_o:

                def emit_av(sexp, h, b, mc, po):
                    """AV with ones-column -> out^T [65, N]; row 64 = denom;
                    normalize and write straight into SBUF attnT."""
                    t0_ = b * NP
                    pso = pp_o.tile([128, 640], FP32, tag="ps_o", name="ps_o")
                    for j in range(MT):
                        mw = 128 if j < MT - 1 else N - 512
                        vj = v_s[:mw, MT * b + j, h, :]
                        nc.tensor.matmul(pso[0:65, 0:512], lhsT=vj,
                                         rhs=sexp[:mw, j, 0:512],
                                         start=(j == 0), stop=(j == MT - 1))
                        nc.tensor.matmul(pso[0:65, 512:N], lhsT=vj,
                                         rhs=sexp[:mw, j, 512:N],
                                         start=(j == 0), stop=(j == MT - 1))
                    # reciprocal of denominator row; replicate across
                    # partitions on GPSIMD (POOL engine is idle)
                    rrow = p_sr.tile([1, 608], FP32, tag="rrow", name="rrow")
                    nc.vector.reciprocal(rrow[:, 0:N], pso[64:65, 0:N])
                    rep = p_sr.tile([64, 608], FP32, tag="rep", name="rep")
                    nc.gpsimd.partition_broadcast(rep[:, 0:N], rrow[:, 0:N])
                    nc.vector.tensor_tensor(attnT[po:po + 64, mc, t0_:t0_ + N],
                                            pso[0:64, 0:N],
                                            rep[:, 0:N], op=ALU.mult)

                pend = None
                for mc in range(CC):
                    # compute Q chunk (heads 2mc, 2mc+1) and K chunk on demand
                    wq_s = p_wqk.tile([128, CC, 128], BF16, tag="wq", name="wq_s")
                    nc.sync.dma_start(wq_s, wqkT_r[:, :, mc * 128:(mc + 1) * 128])
                    wk_s = p_wqk.tile([128, CC, 128], BF16, tag="wk", name="wk_s")
                    nc.sync.dma_start(
                        wk_s, wqkT_r[:, :, C + mc * 128:C + (mc + 1) * 128])
                    qc = p_qk.tile([128, T], BF16, tag="qc", name="qc")
                    kc = p_qk.tile([128, T], BF16, tag="kc", name="kc")
                    for dst, w_s in ((qc, wq_s), (kc, wk_s)):
                        for n5 in range(T // 512):
                            ps = pp_qk.tile([128, 512], FP32, tag="ps_qk",
                                            name="ps_qk")
                            for cc in range(CC):
                                nc.tensor.matmul(
                                    ps, lhsT=w_s[:, cc, :],
                                    rhs=y1T[:, cc, n5 * 512:(n5 + 1) * 512],
                                    start=(cc == 0), stop=(cc == CC - 1))
                            nc.vector.tensor_copy(
                                dst[:, n5 * 512:(n5 + 1) * 512], ps)
                    for h in (2 * mc, 2 * mc + 1):
                        po = (h % 2) * 64
                        for b in range(BPC):
                            t0_ = b * NP
                            QT = qc[po:po + 64, t0_:t0_ + N]
                            sexp = p_se.tile([128, MT, 640], BF16, tag="sexp",
                                             name="sexp")
                            for j in range(MT):
                                mw = 128 if j < MT - 1 else N - 512
                                KTj = kc[po:po + 64,
                                         t0_ + j * 128: t0_ + j * 128 + mw]
                                ps = pp_s.tile([128, 640], FP32, tag="ps_s",
                                               name="ps_s")
                                nc.tensor.matmul(ps[:mw, 0:512], lhsT=KTj,
                                                 rhs=QT[:, 0:512], start=True,
                                                 stop=True)
                                nc.tensor.matmul(ps[:mw, 512:N], lhsT=KTj,
                                                 rhs=QT[:, 512:N], start=True,
                                                 stop=True)
                                nc.scalar.activation(sexp[:mw, j, 0:N],
                                                     ps[:mw, 0:N],
                                                     AF.Exp, scale=SCALE)
                            # software pipeline: emit previous (h,b)'s AV now,
                            # giving its exps a full S-block of slack
                            if pend is not None:
                                emit_av(*pend)
                            pend = (sexp, h, b, mc, po)
                emit_av(*pend)

            # ===== Phase 5: proj + residual + LN2 + transpose =====
            for k in range(2):
                nc.sync.dma_start(wf1a[:, :, k * 512:(k + 1) * 512],
                                  wf1T_r[:, :, k * 512:(k + 1) * 512])
            with tc.tile_pool(name="s5", bufs=3) as s5, \
                 tc.tile_pool(name="s5p", bufs=4, space="PSUM") as s5p, \
                 tc.tile_pool(name="s5pt", bufs=2, space="PSUM") as s5pt:
                BP = load_row("c", bpj)
                for t in range(TT):
                    xr = s5.tile([128, C], FP32, tag="xr", name="xr")
                    nc.gpsimd.dma_start(xr, xp_r[t])
                    x2_t = s5.tile([128, C], FP32, tag="x2", name="x2_t")
                    for n2 in range(2):
                        ps = s5p.tile([128, 512], FP32, tag="ps_p", name="ps_p")
                        for cc in range(CC):
                            nc.tensor.matmul(
                                ps, lhsT=attnT[:, cc, t * 128:(t + 1) * 128],
                                             rhs=wpT_s[:, cc, n2 * 512:(n2 + 1) * 512],
                                             start=(cc == 0), stop=(cc == CC - 1))
                        sl = slice(n2 * 512, (n2 + 1) * 512)
                        nc.vector.tensor_tensor(x2_t[:, sl], ps, xr[:, sl],
                                                op=ALU.add)
                        nc.gpsimd.tensor_tensor(x2_t[:, sl], x2_t[:, sl],
                                                BP[:, sl], op=ALU.add)
                    nc.sync.dma_start(x2d_r[t], x2_t)
                    y2 = layernorm(x2_t, s5)
                    y2Ts = s5.tile([128, CC, 128], BF16, tag="y2Ts", name="y2Ts")
                    transpose_tile(y2, y2Ts, s5pt, "pst2", g2c, b2c)
                    nc.sync.dma_start(y2Td_r[:, :, t * 128:(t + 1) * 128],
                                      y2Ts)

        # ================= Phase 6: MLP (compact token space) ==========
        # the MLP runs over the 2308 real tokens only; compact index
        # u = 577*b + i maps to padded column 640*b + i.
        NREAL = BPC * N
        CHN = list(range(0, NREAL, CH)) + [NREAL]

        def compact_segments(u0, u1):
            segs = []
            while u0 < u1:
                b, i = u0 // N, u0 % N
                take = min(u1 - u0, N - i)
                segs.append((b * NP + i, take))
                u0 += take
            return segs

        xp_f = x2d[:]
        out_f = out[:]
        BF2 = load_row("c", bf2)
        with tc.tile_pool(name="wmlp", bufs=1) as p_wm, \
             tc.tile_pool(name="hT", bufs=1) as p_hT, \
             tc.tile_pool(name="s6", bufs=2) as s6, \
             tc.tile_pool(name="s6p1", bufs=3, space="PSUM") as s6p1, \
             tc.tile_pool(name="s6p2", bufs=4, space="PSUM") as s6p2:
            # chunked weight loads on both HWDGE queues (SP + ACT) so the
            # first fc1 matmuls start ~3us after phase 5 instead of ~50us
            wf1T_s = p_wm.tile([128, CC, HID - 1024], BF16)
            for k in range(6):
                nc.scalar.dma_start(wf1T_s[:, :, k * 512:(k + 1) * 512],
                                    wf1T_r[:, :, 1024 + k * 512:1024 + (k + 1) * 512])
            wf2T_s = p_wm.tile([128, HC, C], BF16)
            wf2T_r = wf2T[:].rearrange("(o p) m -> p o m", p=128)
            for k in range(4):
                nc.sync.dma_start(wf2T_s[:, :, k * 256:(k + 1) * 256],
                                  wf2T_r[:, :, k * 256:(k + 1) * 256])
            for u in range(len(CHN) - 1):
                u0, u1 = CHN[u], CHN[u + 1]
                cw = u1 - u0
                y2c = s6.tile([128, CC, CH], BF16, tag="y2c", name="y2c")
                off = 0
                for pc, ln in compact_segments(u0, u1):
                    nc.gpsimd.dma_start(y2c[:, :, off:off + ln],
                                        y2Td_r[:, :, pc:pc + ln])
                    off += ln
                hT = p_hT.tile([128, HC, CH], BF16, tag="hT", name="hT")
                for hc in range(HC):
                    psf = s6p1.tile([128, CH], FP32, tag="ps_f1", name="ps_f1")
                    if hc < 8:
                        wsrc = wf1a[:, :, hc * 128:(hc + 1) * 128]
                    else:
                        wsrc = wf1T_s[:, :, (hc - 8) * 128:(hc - 7) * 128]
                    for cc in range(CC):
                        nc.tensor.matmul(psf[:, :cw], lhsT=wsrc[:, cc],
                                         rhs=y2c[:, cc, :cw],
                                         start=(cc == 0), stop=(cc == CC - 1))
                    nc.scalar.activation(hT[:, hc, :cw], psf[:, :cw], AF.Gelu,
                                         bias=bf1_s[:, hc:hc + 1])
                for tt_ in range((cw + 127) // 128):
                    m0 = tt_ * 128
                    mw2 = min(128, cw - m0)
                    segs = compact_segments(u0 + m0, u0 + m0 + mw2)
                    xr2 = s6.tile([128, C], FP32, tag="xr2", name="xr2")
                    soff = 0
                    for pc, ln in segs:
                        nc.gpsimd.dma_start(xr2[soff:soff + ln, :],
                                            xp_f[pc:pc + ln, :])
                        soff += ln
                    out_t = s6.tile([128, C], FP32, tag="out", name="out_t")
                    for n2 in range(2):
                        ps2 = s6p2.tile([128, 512], FP32, tag="ps_f2", name="ps_f2")
                        for hc in range(HC):
                            nc.tensor.matmul(
                                ps2[:mw2], lhsT=hT[:, hc, m0:m0 + mw2],
                                rhs=wf2T_s[:, hc, n2 * 512:(n2 + 1) * 512],
                                start=(hc == 0), stop=(hc == HC - 1))
                        sl = slice(n2 * 512, (n2 + 1) * 512)
                        nc.vector.tensor_tensor(out_t[:mw2, sl], ps2[:mw2],
                                                BF2[:mw2, sl], op=ALU.add)
                        nc.vector.tensor_tensor(out_t[:mw2, sl], out_t[:mw2, sl],
                                                xr2[:mw2, sl], op=ALU.add)
                    soff = 0
                    for pc, ln in segs:
                        nc.sync.dma_start(out_f[pc:pc + ln, :],
                                          out_t[soff:soff + ln, :])
                        soff += ln


def _build(nc, reps=1):
    io = _declare_io(nc)
    with tile.TileContext(nc) as tc:
        for _rep in range(reps):
            _build_once(nc, tc, io)
    return nc


_NC_CACHE = {}


def _get_nc(reps=1):
    if reps not in _NC_CACHE:
        nc = bacc.Bacc(None, target_bir_lowering=False)
        _build(nc, reps=reps)
        nc.compile()
        _NC_CACHE[reps] = nc
    return _NC_CACHE[reps]


def kernel(x, ln1_g, ln1_b, w_qkv, w_proj, b_proj, ln2_g, ln2_b,
           w_fc1, b_fc1, w_fc2, b_fc2, _trace=False, _trace_kwargs=None):
    nc = _get_nc()

    def bf(a):
        return np.ascontiguousarray(np.asarray(a, np.float32).T).astype(BF16NP)

    x = np.asarray(x, np.float32)
    shared = {
        "wqkT": bf(w_qkv[:2 * C]),
        "wvT": bf(w_qkv[2 * C:]),
        "wpT": bf(w_proj),
        "wf1T": bf(w_fc1),
        "wf2T": bf(w_fc2),
        "g1": np.asarray(ln1_g, np.float32),
        "b1": np.asarray(ln1_b, np.float32),
        "g2": np.asarray(ln2_g, np.float32),
        "b2": np.asarray(ln2_b, np.float32),
        "bpj": np.asarray(b_proj, np.float32),
        "bf1": np.asarray(b_fc1, np.float32),
        "bf2": np.asarray(b_fc2, np.float32),
    }
    xs = x.reshape(NCORES, BPC, N, C)
    xpad = np.zeros((NCORES, BPC, NP, C), np.float32)
    xpad[:, :, :N] = xs
    in_maps = [dict(shared, xp=np.ascontiguousarray(xpad[c].reshape(T, C)))
               for c in range(NCORES)]

    kw = {}
    if _trace:
        kw = dict(trace=True, trace_kwargs=_trace_kwargs or {})
    res = bass_utils.run_bass_kernel_spmd(nc, in_maps, core_ids=list(range(NCORES)),
                                          **kw)
    kernel.last_results = res
    outs = []
    for c in range(NCORES):
        oc = np.asarray(res.results[c]["out"]).reshape(BPC, NP, C)[:, :N]
        outs.append(oc)
    return np.concatenate(outs, axis=0).astype(np.float32)


# revision 30
# speedup vs baseline: 1.0122x; 1.0122x over previous
# Trainium2 Bass kernel for a ViT-style transformer block.
#   x = x + proj(attn(LN1(x)));  x = x + fc2(gelu(fc1(LN2(x))))
# B=32, N=577, C=1024, H=16, D=64, HID=4096.
#
# Distribution: pure data-parallel over batch, 4 images per NeuronCore.
# Per-core token layout: each image padded 577 -> 640 tokens, so a core
# processes T = 4*640 = 2560 tokens = 20 tiles of 128.
#
# All matmuls run in bf16 (fp32 PSUM accumulation); LN stats, softmax
# denominators and the residual stream stay fp32.
#
# Attention is computed keys-major: S^T = K_T^T @ Q_T per (b,h), exp on
# ScalarE (no max subtraction -- logits are O(1) here), AV as
# out^T = V_ext^T @ P^T where V_ext carries an extra ones column whose
# output row is the softmax denominator; normalization multiplies by a
# reciprocal row replicated across partitions on GPSIMD.  Activations
# cross phases feature-major (features on partitions) so matmul
# contractions never need activation transposes except the two LN
# outputs (PE transpose).  Q/K projection chunks are computed on demand
# inside the attention loop (2 head's worth at a time) which overlaps
# projection matmuls with attention and keeps SBUF small.

import numpy as np
import ml_dtypes

import concourse.bass as bass
import concourse.mybir as mybir
import concourse.tile as tile
from concourse import bacc
from concourse import bass_utils
from concourse.masks import make_identity

B, N, C = 32, 577, 1024
H, D = 16, 64
HID = 4 * C
EPS = 1e-5
SCALE = D ** -0.5

NCORES = 8
BPC = B // NCORES          # batches per core
NP = 640                   # padded tokens per batch (5 * 128)
T = BPC * NP               # 2560 padded tokens per core
TT = T // 128              # 20 token tiles
CC = C // 128              # 8 feature chunks
MT = NP // 128             # 5 key tiles per batch
HC = HID // 128            # 32 hidden chunks
CH = 512                   # MLP token chunk
NU = T // CH               # 10 MLP chunks

FP32 = mybir.dt.float32
BF16 = mybir.dt.bfloat16
AF = mybir.ActivationFunctionType
ALU = mybir.AluOpType
BF16NP = ml_dtypes.bfloat16


def _declare_io(nc):
    xp = nc.dram_tensor("xp", [T, C], FP32, kind="ExternalInput")
    wqkT = nc.dram_tensor("wqkT", [C, 2 * C], BF16, kind="ExternalInput")
    wvT = nc.dram_tensor("wvT", [C, C], BF16, kind="ExternalInput")
    wpT = nc.dram_tensor("wpT", [C, C], BF16, kind="ExternalInput")
    wf1T = nc.dram_tensor("wf1T", [C, HID], BF16, kind="ExternalInput")
    wf2T = nc.dram_tensor("wf2T", [HID, C], BF16, kind="ExternalInput")
    g1 = nc.dram_tensor("g1", [C], FP32, kind="ExternalInput")
    b1 = nc.dram_tensor("b1", [C], FP32, kind="ExternalInput")
    g2 = nc.dram_tensor("g2", [C], FP32, kind="ExternalInput")
    b2 = nc.dram_tensor("b2", [C], FP32, kind="ExternalInput")
    bpj = nc.dram_tensor("bpj", [C], FP32, kind="ExternalInput")
    bf1 = nc.dram_tensor("bf1", [HID], FP32, kind="ExternalInput")
    bf2 = nc.dram_tensor("bf2", [C], FP32, kind="ExternalInput")
    out = nc.dram_tensor("out", [T, C], FP32, kind="ExternalOutput")
    # DRAM scratch
    x2d = nc.dram_tensor("x2d", [T, C], FP32, kind="Internal")
    y2Td = nc.dram_tensor("y2Td", [C, T], BF16, kind="Internal")
    return (xp, wqkT, wvT, wpT, wf1T, wf2T, g1, b1, g2, b2, bpj, bf1, bf2,
            out, x2d, y2Td)


def _build_once(nc, tc, io):
    (xp, wqkT, wvT, wpT, wf1T, wf2T, g1, b1, g2, b2, bpj, bf1, bf2,
     out, x2d, y2Td) = io
    xp_r = xp[:].rearrange("(t p) c -> t p c", p=128)
    out_r = out[:].rearrange("(t p) c -> t p c", p=128)
    x2d_r = x2d[:].rearrange("(t p) c -> t p c", p=128)
    y2Td_r = y2Td[:].rearrange("(o p) t -> p o t", p=128)
    wqkT_r = wqkT[:].rearrange("(o p) m -> p o m", p=128)

    def bcast_row(ap1d, parts=128):
        # DRAM [C] -> [parts, C] partition-broadcast AP
        return bass.AP(tensor=ap1d.tensor, offset=ap1d.offset,
                       ap=[[0, parts]] + list(ap1d.ap))

    with tc.tile_pool(name="const", bufs=1) as const, \
         tc.tile_pool(name="aff", bufs=1) as aff:
        eps_t = const.tile([128, 1], FP32)
        nc.vector.memset(eps_t, EPS)
        ident = const.tile([128, 128], FP32)
        make_identity(nc, ident)
        bf1_s = const.tile([128, HC], FP32)
        nc.scalar.dma_start(bf1_s, bf1[:].rearrange("(o p) -> p o", p=128))
        # LN gains/biases feature-major [128, CC]: applied per-partition at
        # the transpose eviction (features sit on partitions there)
        g1c = const.tile([128, CC], FP32)
        nc.scalar.dma_start(g1c, g1[:].rearrange("(o p) -> p o", p=128))
        b1c = const.tile([128, CC], FP32)
        nc.scalar.dma_start(b1c, b1[:].rearrange("(o p) -> p o", p=128))
        g2c = const.tile([128, CC], FP32)
        nc.scalar.dma_start(g2c, g2[:].rearrange("(o p) -> p o", p=128))
        b2c = const.tile([128, CC], FP32)
        nc.scalar.dma_start(b2c, b2[:].rearrange("(o p) -> p o", p=128))
        # staging area for the first quarter of fc1 weights: loaded during
        # phase 5 so the MLP's first matmuls start with zero DMA wait
        wf1a = const.tile([128, CC, 1024], BF16)
        wf1T_r = wf1T[:].rearrange("(o p) m -> p o m", p=128)

        def load_row(name, src):
            t_ = aff.tile([128, C], FP32, tag=name, name=name + "_row")
            nc.sync.dma_start(t_, bcast_row(src[:]))
            return t_

        def layernorm(x_t, pool_tmp):
            """x_t [128,C] fp32 -> returns [128,C] fp32 = (x - mu) * rstd"""
            st = pool_tmp.tile([128, 2, 6], FP32, tag="bnst", name="st")
            nc.vector.bn_stats(st[:, 0], x_t[:, 0:512])
            nc.vector.bn_stats(st[:, 1], x_t[:, 512:1024])
            mv = pool_tmp.tile([128, 2], FP32, tag="bnmv", name="mv")
            nc.vector.bn_aggr(mv, st)
            rstd = pool_tmp.tile([128, 1], FP32, tag="rstd", name="rstd")
            nc.scalar.activation(rstd, mv[:, 1:2], AF.Sqrt, bias=eps_t)
            nc.vector.reciprocal(rstd, rstd)
            t0 = pool_tmp.tile([128, C], FP32, tag="t0", name="t0")
            nc.vector.tensor_scalar(t0, x_t, scalar1=mv[:, 0:1], scalar2=rstd,
                                    op0=ALU.subtract, op1=ALU.mult)
            return t0

        def transpose_tile(y_t, dst_ap, pool_ps, tag, gc, bc):
            """y_t [128, C] fp32 -> dst_ap [128, CC, 128] bf16 feature-major
            strip; the eviction applies the per-feature LN gain/bias, which
            are per-PARTITION scalars after the transpose."""
            pt = pool_ps.tile([128, CC, 128], FP32, tag=tag, name=tag)
            for cc in range(CC):
                nc.tensor.transpose(pt[:, cc, :], y_t[:, cc * 128:(cc + 1) * 128],
                                    ident)
            for cc in range(CC):
                nc.scalar.activation(dst_ap[:, cc], pt[:, cc], AF.Identity,
                                     bias=bc[:, cc:cc + 1], scale=gc[:, cc:cc + 1])

        with tc.tile_pool(name="wp", bufs=1) as p_wp, \
             tc.tile_pool(name="attnT", bufs=1) as p_aT, \
             tc.tile_pool(name="v", bufs=1) as p_v, \
             tc.tile_pool(name="y1T", bufs=1) as p_y1T:
            attnT = p_aT.tile([128, CC, T], BF16)
            v_s = p_v.tile([128, TT, H, 65], BF16)
            nc.vector.memset(v_s[:, :, :, 64:65], 1.0)
            y1T = p_y1T.tile([128, CC, T], BF16)

            # ===== Phase 1+2 fused: LN1 + transpose + V projection =====
            with tc.tile_pool(name="wv", bufs=1) as p_wv, \
                 tc.tile_pool(name="s1", bufs=3) as s1, \
                 tc.tile_pool(name="s1p", bufs=2, space="PSUM") as s1p, \
                 tc.tile_pool(name="s2p", bufs=4, space="PSUM") as s2p:
                wvT_s = p_wv.tile([128, CC, C], BF16)
                nc.scalar.dma_start(wvT_s, wvT[:].rearrange("(o p) m -> p o m", p=128))
                for t in range(TT):
                    x_t = s1.tile([128, C], FP32, tag="x", name="x_t")
                    nc.sync.dma_start(x_t, xp_r[t])
                    y1 = layernorm(x_t, s1)
                    transpose_tile(y1, y1T[:, :, t * 128:(t + 1) * 128], s1p,
                                   "pst1", g1c, b1c)
                    for n2 in range(2):
                        ps = s2p.tile([128, 512], FP32, tag="ps_v", name="ps_v")
                        for cc in range(CC):
                            nc.tensor.matmul(ps,
                                             lhsT=y1T[:, cc, t * 128:(t + 1) * 128],
                                             rhs=wvT_s[:, cc, n2 * 512:(n2 + 1) * 512],
                                             start=(cc == 0), stop=(cc == CC - 1))
                        nc.scalar.copy(v_s[:, t, n2 * 8:(n2 + 1) * 8, 0:64],
                                       ps.rearrange("p (h d) -> p h d", h=8))

            # ===== Phase 3+4 fused: per head-pair QK projection + attention ===
            # prefetch proj weights (needed from phase 5)
            wpT_s = p_wp.tile([128, CC, C], BF16)
            nc.sync.dma_start(wpT_s, wpT[:].rearrange("(o p) m -> p o m", p=128))
            with tc.tile_pool(name="qk", bufs=2) as p_qk, \
                 tc.tile_pool(name="wqk", bufs=2) as p_wqk, \
                 tc.tile_pool(name="sexp", bufs=2) as p_se, \
                 tc.tile_pool(name="srow", bufs=2) as p_sr, \
                 tc.tile_pool(name="ps_qk", bufs=2, space="PSUM") as pp_qk, \
                 tc.tile_pool(name="ps_s", bufs=2, space="PSUM") as pp_s, \
                 tc.tile_pool(name="ps_o", bufs=1, space="PSUM") as pp_o:

                def emit_av(sexp, h, b, mc, po):
                    """AV with ones-column -> out^T [65, N]; row 64 = denom;
                    normalize and write straight into SBUF attnT."""
                    t0_ = b * NP
                    pso = pp_o.tile([128, 640], FP32, tag="ps_o", name="ps_o")
                    for j in range(MT):
                        mw = 128 if j < MT - 1 else N - 512
                        vj = v_s[:mw, MT * b + j, h, :]
                        nc.tensor.matmul(pso[0:65, 0:512], lhsT=vj,
                                         rhs=sexp[:mw, j, 0:512],
                                         start=(j == 0), stop=(j == MT - 1))
                        nc.tensor.matmul(pso[0:65, 512:N], lhsT=vj,
                                         rhs=sexp[:mw, j, 512:N],
                                         start=(j == 0), stop=(j == MT - 1))
                    # reciprocal of denominator row; replicate across
                    # partitions on GPSIMD (POOL engine is idle)
                    rrow = p_sr.tile([1, 608], FP32, tag="rrow", name="rrow")
                    nc.vector.reciprocal(rrow[:, 0:N], pso[64:65, 0:N])
                    rep = p_sr.tile([64, 608], FP32, tag="rep", name="rep")
                    nc.gpsimd.partition_broadcast(rep[:, 0:N], rrow[:, 0:N])
                    nc.vector.tensor_tensor(attnT[po:po + 64, mc, t0_:t0_ + N],
                                            pso[0:64, 0:N],
                                            rep[:, 0:N], op=ALU.mult)

                # compact token space for Q/K: u = 577*b + i <-> padded
                # column 640*b + i
                NREALA = BPC * N
                QCH = list(range(0, NREALA, 512)) + [NREALA]

                def qsegs(u0, u1):
                    segs = []
                    while u0 < u1:
                        b, i = u0 // N, u0 % N
                        take = min(u1 - u0, N - i)
                        segs.append((b * NP + i, take))
                        u0 += take
                    return segs

                pend = None
                for mc in range(CC):
                    # compute Q chunk (heads 2mc, 2mc+1) and K chunk on demand
                    wq_s = p_wqk.tile([128, CC, 128], BF16, tag="wq", name="wq_s")
                    nc.sync.dma_start(wq_s, wqkT_r[:, :, mc * 128:(mc + 1) * 128])
                    wk_s = p_wqk.tile([128, CC, 128], BF16, tag="wk", name="wk_s")
                    nc.sync.dma_start(
                        wk_s, wqkT_r[:, :, C + mc * 128:C + (mc + 1) * 128])
                    qc = p_qk.tile([128, T], BF16, tag="qc", name="qc")
                    kc = p_qk.tile([128, T], BF16, tag="kc", name="kc")
                    for dst, w_s in ((qc, wq_s), (kc, wk_s)):
                        for n5 in range(len(QCH) - 1):
                            u0, u1 = QCH[n5], QCH[n5 + 1]
                            cw = u1 - u0
                            ps = pp_qk.tile([128, 512], FP32, tag="ps_qk",
                                            name="ps_qk")
                            off = 0
                            for pc, ln in qsegs(u0, u1):
                                for cc in range(CC):
                                    nc.tensor.matmul(
                                        ps[:, off:off + ln], lhsT=w_s[:, cc, :],
                                        rhs=y1T[:, cc, pc:pc + ln],
                                        start=(cc == 0), stop=(cc == CC - 1))
                                off += ln
                            nc.vector.tensor_copy(dst[:, u0:u0 + cw],
                                                  ps[:, 0:cw])
                    for h in (2 * mc, 2 * mc + 1):
                        po = (h % 2) * 64
                        for b in range(BPC):
                            t0_ = b * NP
                            tq_ = b * N
                            QT = qc[po:po + 64, tq_:tq_ + N]
                            sexp = p_se.tile([128, MT, 640], BF16, tag="sexp",
                                             name="sexp")
                            for j in range(MT):
                                mw = 128 if j < MT - 1 else N - 512
                                KTj = kc[po:po + 64,
                                         tq_ + j * 128: tq_ + j * 128 + mw]
                                ps = pp_s.tile([128, 640], FP32, tag="ps_s",
                                               name="ps_s")
                                nc.tensor.matmul(ps[:mw, 0:512], lhsT=KTj,
                                                 rhs=QT[:, 0:512], start=True,
                                                 stop=True)
                                nc.tensor.matmul(ps[:mw, 512:N], lhsT=KTj,
                                                 rhs=QT[:, 512:N], start=True,
                                                 stop=True)
                                nc.scalar.activation(sexp[:mw, j, 0:N],
                                                     ps[:mw, 0:N],
                                                     AF.Exp, scale=SCALE)
                            # software pipeline: emit previous (h,b)'s AV now,
                            # giving its exps a full S-block of slack
                            if pend is not None:
                                emit_av(*pend)
                            pend = (sexp, h, b, mc, po)
                emit_av(*pend)

            # ===== Phase 5: proj + residual + LN2 + transpose =====
            for k in range(2):
                nc.sync.dma_start(wf1a[:, :, k * 512:(k + 1) * 512],
                                  wf1T_r[:, :, k * 512:(k + 1) * 512])
            with tc.tile_pool(name="s5", bufs=3) as s5, \
                 tc.tile_pool(name="s5p", bufs=4, space="PSUM") as s5p, \
                 tc.tile_pool(name="s5pt", bufs=2, space="PSUM") as s5pt:
                BP = load_row("c", bpj)
                for t in range(TT):
                    xr = s5.tile([128, C], FP32, tag="xr", name="xr")
                    nc.gpsimd.dma_start(xr, xp_r[t])
                    x2_t = s5.tile([128, C], FP32, tag="x2", name="x2_t")
                    for n2 in range(2):
                        ps = s5p.tile([128, 512], FP32, tag="ps_p", name="ps_p")
                        for cc in range(CC):
                            nc.tensor.matmul(
                                ps, lhsT=attnT[:, cc, t * 128:(t + 1) * 128],
                                             rhs=wpT_s[:, cc, n2 * 512:(n2 + 1) * 512],
                                             start=(cc == 0), stop=(cc == CC - 1))
                        sl = slice(n2 * 512, (n2 + 1) * 512)
                        nc.vector.tensor_tensor(x2_t[:, sl], ps, xr[:, sl],
                                                op=ALU.add)
                        nc.gpsimd.tensor_tensor(x2_t[:, sl], x2_t[:, sl],
                                                BP[:, sl], op=ALU.add)
                    nc.sync.dma_start(x2d_r[t], x2_t)
                    y2 = layernorm(x2_t, s5)
                    y2Ts = s5.tile([128, CC, 128], BF16, tag="y2Ts", name="y2Ts")
                    transpose_tile(y2, y2Ts, s5pt, "pst2", g2c, b2c)
                    nc.sync.dma_start(y2Td_r[:, :, t * 128:(t + 1) * 128],
                                      y2Ts)

        # ================= Phase 6: MLP (compact token space) ==========
        # the MLP runs over the 2308 real tokens only; compact index
        # u = 577*b + i maps to padded column 640*b + i.
        NREAL = BPC * N
        CHN = list(range(0, NREAL, CH)) + [NREAL]

        def compact_segments(u0, u1):
            segs = []
            while u0 < u1:
                b, i = u0 // N, u0 % N
                take = min(u1 - u0, N - i)
                segs.append((b * NP + i, take))
                u0 += take
            return segs

        xp_f = x2d[:]
        out_f = out[:]
        BF2 = load_row("c", bf2)
        with tc.tile_pool(name="wmlp", bufs=1) as p_wm, \
             tc.tile_pool(name="hT", bufs=1) as p_hT, \
             tc.tile_pool(name="s6", bufs=2) as s6, \
             tc.tile_pool(name="s6p1", bufs=4, space="PSUM") as s6p1, \
             tc.tile_pool(name="s6p2", bufs=4, space="PSUM") as s6p2:
            # chunked weight loads on both HWDGE queues (SP + ACT) so the
            # first fc1 matmuls start ~3us after phase 5 instead of ~50us
            wf1T_s = p_wm.tile([128, CC, HID - 1024], BF16)
            for k in range(6):
                nc.scalar.dma_start(wf1T_s[:, :, k * 512:(k + 1) * 512],
                                    wf1T_r[:, :, 1024 + k * 512:1024 + (k + 1) * 512])
            wf2T_s = p_wm.tile([128, HC, C], BF16)
            wf2T_r = wf2T[:].rearrange("(o p) m -> p o m", p=128)
            for k in range(4):
                nc.sync.dma_start(wf2T_s[:, :, k * 256:(k + 1) * 256],
                                  wf2T_r[:, :, k * 256:(k + 1) * 256])
            for u in range(len(CHN) - 1):
                u0, u1 = CHN[u], CHN[u + 1]
                cw = u1 - u0
                y2c = s6.tile([128, CC, CH], BF16, tag="y2c", name="y2c")
                off = 0
                for pc, ln in compact_segments(u0, u1):
                    nc.gpsimd.dma_start(y2c[:, :, off:off + ln],
                                        y2Td_r[:, :, pc:pc + ln])
                    off += ln
                hT = p_hT.tile([128, HC, CH], BF16, tag="hT", name="hT")
                for hc in range(HC):
                    psf = s6p1.tile([128, CH], FP32, tag="ps_f1", name="ps_f1")
                    if hc < 8:
                        wsrc = wf1a[:, :, hc * 128:(hc + 1) * 128]
                    else:
                        wsrc = wf1T_s[:, :, (hc - 8) * 128:(hc - 7) * 128]
                    for cc in range(CC):
                        nc.tensor.matmul(psf[:, :cw], lhsT=wsrc[:, cc],
                                         rhs=y2c[:, cc, :cw],
                                         start=(cc == 0), stop=(cc == CC - 1))
                    nc.scalar.activation(hT[:, hc, :cw], psf[:, :cw], AF.Gelu,
                                         bias=bf1_s[:, hc:hc + 1])
                for tt_ in range((cw + 127) // 128):
                    m0 = tt_ * 128
                    mw2 = min(128, cw - m0)
                    segs = compact_segments(u0 + m0, u0 + m0 + mw2)
                    xr2 = s6.tile([128, C], FP32, tag="xr2", name="xr2")
                    soff = 0
                    for pc, ln in segs:
                        nc.gpsimd.dma_start(xr2[soff:soff + ln, :],
                                            xp_f[pc:pc + ln, :])
                        soff += ln
                    out_t = s6.tile([128, C], FP32, tag="out", name="out_t")
                    for n2 in range(2):
                        ps2 = s6p2.tile([128, 512], FP32, tag="ps_f2", name="ps_f2")
                        for hc in range(HC):
                            nc.tensor.matmul(
                                ps2[:mw2], lhsT=hT[:, hc, m0:m0 + mw2],
                                rhs=wf2T_s[:, hc, n2 * 512:(n2 + 1) * 512],
                                start=(hc == 0), stop=(hc == HC - 1))
                        sl = slice(n2 * 512, (n2 + 1) * 512)
                        nc.vector.tensor_tensor(out_t[:mw2, sl], ps2[:mw2],
                                                BF2[:mw2, sl], op=ALU.add)
                        nc.vector.tensor_tensor(out_t[:mw2, sl], out_t[:mw2, sl],
                                                xr2[:mw2, sl], op=ALU.add)
                    soff = 0
                    for pc, ln in segs:
                        nc.sync.dma_start(out_f[pc:pc + ln, :],
                                          out_t[soff:soff + ln, :])
                        soff += ln


def _build(nc, reps=1):
    io = _declare_io(nc)
    with tile.TileContext(nc) as tc:
        for _rep in range(reps):
            _build_once(nc, tc, io)
    return nc


_NC_CACHE = {}


def _get_nc(reps=1):
    if reps not in _NC_CACHE:
        nc = bacc.Bacc(None, target_bir_lowering=False)
        _build(nc, reps=reps)
        nc.compile()
        _NC_CACHE[reps] = nc
    return _NC_CACHE[reps]


def kernel(x, ln1_g, ln1_b, w_qkv, w_proj, b_proj, ln2_g, ln2_b,
           w_fc1, b_fc1, w_fc2, b_fc2, _trace=False, _trace_kwargs=None):
    nc = _get_nc()

    def bf(a):
        return np.ascontiguousarray(np.asarray(a, np.float32).T).astype(BF16NP)

    x = np.asarray(x, np.float32)
    shared = {
        "wqkT": bf(w_qkv[:2 * C]),
        "wvT": bf(w_qkv[2 * C:]),
        "wpT": bf(w_proj),
        "wf1T": bf(w_fc1),
        "wf2T": bf(w_fc2),
        "g1": np.asarray(ln1_g, np.float32),
        "b1": np.asarray(ln1_b, np.float32),
        "g2": np.asarray(ln2_g, np.float32),
        "b2": np.asarray(ln2_b, np.float32),
        "bpj": np.asarray(b_proj, np.float32),
        "bf1": np.asarray(b_fc1, np.float32),
        "bf2": np.asarray(b_fc2, np.float32),
    }
    xs = x.reshape(NCORES, BPC, N, C)
    xpad = np.zeros((NCORES, BPC, NP, C), np.float32)
    xpad[:, :, :N] = xs
    in_maps = [dict(shared, xp=np.ascontiguousarray(xpad[c].reshape(T, C)))
               for c in range(NCORES)]

    kw = {}
    if _trace:
        kw = dict(trace=True, trace_kwargs=_trace_kwargs or {})
    res = bass_utils.run_bass_kernel_spmd(nc, in_maps, core_ids=list(range(NCORES)),
                                          **kw)
    kernel.last_results = res
    outs = []
    for c in range(NCORES):
        oc = np.asarray(res.results[c]["out"]).reshape(BPC, NP, C)[:, :N]
        outs.append(oc)
    return np.concatenate(outs, axis=0).astype(np.float32)


# revision 31
# speedup vs baseline: 1.0153x; 1.0030x over previous
# Trainium2 Bass kernel for a ViT-style transformer block.
#   x = x + proj(attn(LN1(x)));  x = x + fc2(gelu(fc1(LN2(x))))
# B=32, N=577, C=1024, H=16, D=64, HID=4096.
#
# Distribution: pure data-parallel over batch, 4 images per NeuronCore.
# Per-core token layout: each image padded 577 -> 640 tokens, so a core
# processes T = 4*640 = 2560 tokens = 20 tiles of 128.
#
# All matmuls run in bf16 (fp32 PSUM accumulation); LN stats, softmax
# denominators and the residual stream stay fp32.
#
# Attention is computed keys-major: S^T = K_T^T @ Q_T per (b,h), exp on
# ScalarE (no max subtraction -- logits are O(1) here), AV as
# out^T = V_ext^T @ P^T where V_ext carries an extra ones column whose
# output row is the softmax denominator; normalization multiplies by a
# reciprocal row replicated across partitions on GPSIMD.  Activations
# cross phases feature-major (features on partitions) so matmul
# contractions never need activation transposes except the two LN
# outputs (PE transpose).  Q/K projection chunks are computed on demand
# inside the attention loop (2 head's worth at a time) which overlaps
# projection matmuls with attention and keeps SBUF small.

import numpy as np
import ml_dtypes

import concourse.bass as bass
import concourse.mybir as mybir
import concourse.tile as tile
from concourse import bacc
from concourse import bass_utils
from concourse.masks import make_identity

B, N, C = 32, 577, 1024
H, D = 16, 64
HID = 4 * C
EPS = 1e-5
SCALE = D ** -0.5

NCORES = 8
BPC = B // NCORES          # batches per core
NP = 640                   # padded tokens per batch (5 * 128)
T = BPC * NP               # 2560 padded tokens per core
TT = T // 128              # 20 token tiles
CC = C // 128              # 8 feature chunks
MT = NP // 128             # 5 key tiles per batch
HC = HID // 128            # 32 hidden chunks
CH = 512                   # MLP token chunk
NU = T // CH               # 10 MLP chunks

FP32 = mybir.dt.float32
BF16 = mybir.dt.bfloat16
AF = mybir.ActivationFunctionType
ALU = mybir.AluOpType
BF16NP = ml_dtypes.bfloat16


def _declare_io(nc):
    xp = nc.dram_tensor("xp", [T, C], FP32, kind="ExternalInput")
    wqkT = nc.dram_tensor("wqkT", [C, 2 * C], BF16, kind="ExternalInput")
    wvT = nc.dram_tensor("wvT", [C, C], BF16, kind="ExternalInput")
    wpT = nc.dram_tensor("wpT", [C, C], BF16, kind="ExternalInput")
    wf1T = nc.dram_tensor("wf1T", [C, HID], BF16, kind="ExternalInput")
    wf2T = nc.dram_tensor("wf2T", [HID, C], BF16, kind="ExternalInput")
    g1 = nc.dram_tensor("g1", [C], FP32, kind="ExternalInput")
    b1 = nc.dram_tensor("b1", [C], FP32, kind="ExternalInput")
    g2 = nc.dram_tensor("g2", [C], FP32, kind="ExternalInput")
    b2 = nc.dram_tensor("b2", [C], FP32, kind="ExternalInput")
    bpj = nc.dram_tensor("bpj", [C], FP32, kind="ExternalInput")
    bf1 = nc.dram_tensor("bf1", [HID], FP32, kind="ExternalInput")
    bf2 = nc.dram_tensor("bf2", [C], FP32, kind="ExternalInput")
    out = nc.dram_tensor("out", [T, C], FP32, kind="ExternalOutput")
    # DRAM scratch
    x2d = nc.dram_tensor("x2d", [T, C], FP32, kind="Internal")
    y2Td = nc.dram_tensor("y2Td", [C, T], BF16, kind="Internal")
    return (xp, wqkT, wvT, wpT, wf1T, wf2T, g1, b1, g2, b2, bpj, bf1, bf2,
            out, x2d, y2Td)


def _build_once(nc, tc, io):
    (xp, wqkT, wvT, wpT, wf1T, wf2T, g1, b1, g2, b2, bpj, bf1, bf2,
     out, x2d, y2Td) = io
    xp_r = xp[:].rearrange("(t p) c -> t p c", p=128)
    out_r = out[:].rearrange("(t p) c -> t p c", p=128)
    x2d_r = x2d[:].rearrange("(t p) c -> t p c", p=128)
    y2Td_r = y2Td[:].rearrange("(o p) t -> p o t", p=128)
    wqkT_r = wqkT[:].rearrange("(o p) m -> p o m", p=128)

    def bcast_row(ap1d, parts=128):
        # DRAM [C] -> [parts, C] partition-broadcast AP
        return bass.AP(tensor=ap1d.tensor, offset=ap1d.offset,
                       ap=[[0, parts]] + list(ap1d.ap))

    with tc.tile_pool(name="const", bufs=1) as const, \
         tc.tile_pool(name="aff", bufs=1) as aff:
        eps_t = const.tile([128, 1], FP32)
        nc.vector.memset(eps_t, EPS)
        ident = const.tile([128, 128], FP32)
        make_identity(nc, ident)
        bf1_s = const.tile([128, HC], FP32)
        nc.scalar.dma_start(bf1_s, bf1[:].rearrange("(o p) -> p o", p=128))
        # LN gains/biases feature-major [128, CC]: applied per-partition at
        # the transpose eviction (features sit on partitions there)
        g1c = const.tile([128, CC], FP32)
        nc.scalar.dma_start(g1c, g1[:].rearrange("(o p) -> p o", p=128))
        b1c = const.tile([128, CC], FP32)
        nc.scalar.dma_start(b1c, b1[:].rearrange("(o p) -> p o", p=128))
        g2c = const.tile([128, CC], FP32)
        nc.scalar.dma_start(g2c, g2[:].rearrange("(o p) -> p o", p=128))
        b2c = const.tile([128, CC], FP32)
        nc.scalar.dma_start(b2c, b2[:].rearrange("(o p) -> p o", p=128))
        # staging area for the first quarter of fc1 weights: loaded during
        # phase 5 so the MLP's first matmuls start with zero DMA wait
        wf1a = const.tile([128, CC, 1024], BF16)
        wf1T_r = wf1T[:].rearrange("(o p) m -> p o m", p=128)

        def load_row(name, src):
            t_ = aff.tile([128, C], FP32, tag=name, name=name + "_row")
            nc.sync.dma_start(t_, bcast_row(src[:]))
            return t_

        def layernorm(x_t, pool_tmp):
            """x_t [128,C] fp32 -> returns [128,C] fp32 = (x - mu) * rstd"""
            st = pool_tmp.tile([128, 2, 6], FP32, tag="bnst", name="st")
            nc.vector.bn_stats(st[:, 0], x_t[:, 0:512])
            nc.vector.bn_stats(st[:, 1], x_t[:, 512:1024])
            mv = pool_tmp.tile([128, 2], FP32, tag="bnmv", name="mv")
            nc.vector.bn_aggr(mv, st)
            rstd = pool_tmp.tile([128, 1], FP32, tag="rstd", name="rstd")
            nc.scalar.activation(rstd, mv[:, 1:2], AF.Sqrt, bias=eps_t)
            nc.vector.reciprocal(rstd, rstd)
            t0 = pool_tmp.tile([128, C], FP32, tag="t0", name="t0")
            nc.vector.tensor_scalar(t0, x_t, scalar1=mv[:, 0:1], scalar2=rstd,
                                    op0=ALU.subtract, op1=ALU.mult)
            return t0

        def transpose_tile(y_t, dst_ap, pool_ps, tag, gc, bc):
            """y_t [128, C] fp32 -> dst_ap [128, CC, 128] bf16 feature-major
            strip; the eviction applies the per-feature LN gain/bias, which
            are per-PARTITION scalars after the transpose."""
            pt = pool_ps.tile([128, CC, 128], FP32, tag=tag, name=tag)
            for cc in range(CC):
                nc.tensor.transpose(pt[:, cc, :], y_t[:, cc * 128:(cc + 1) * 128],
                                    ident)
            for cc in range(CC):
                nc.scalar.activation(dst_ap[:, cc], pt[:, cc], AF.Identity,
                                     bias=bc[:, cc:cc + 1], scale=gc[:, cc:cc + 1])

        with tc.tile_pool(name="wp", bufs=1) as p_wp, \
             tc.tile_pool(name="attnT", bufs=1) as p_aT, \
             tc.tile_pool(name="v", bufs=1) as p_v, \
             tc.tile_pool(name="y1T", bufs=1) as p_y1T:
            attnT = p_aT.tile([128, CC, T], BF16)
            v_s = p_v.tile([128, TT, H, 65], BF16)
            nc.vector.memset(v_s[:, :, :, 64:65], 1.0)
            y1T = p_y1T.tile([128, CC, T], BF16)

            # ===== Phase 1+2 fused: LN1 + transpose + V projection =====
            with tc.tile_pool(name="wv", bufs=1) as p_wv, \
                 tc.tile_pool(name="s1", bufs=3) as s1, \
                 tc.tile_pool(name="s1p", bufs=2, space="PSUM") as s1p, \
                 tc.tile_pool(name="s2p", bufs=4, space="PSUM") as s2p:
                wvT_s = p_wv.tile([128, CC, C], BF16)
                wvT_r2 = wvT[:].rearrange("(o p) m -> p o m", p=128)
                for k in range(2):
                    nc.scalar.dma_start(wvT_s[:, :, k * 512:(k + 1) * 512],
                                        wvT_r2[:, :, k * 512:(k + 1) * 512])
                for t in range(TT):
                    x_t = s1.tile([128, C], FP32, tag="x", name="x_t")
                    nc.sync.dma_start(x_t, xp_r[t])
                    y1 = layernorm(x_t, s1)
                    transpose_tile(y1, y1T[:, :, t * 128:(t + 1) * 128], s1p,
                                   "pst1", g1c, b1c)
                    for n2 in range(2):
                        ps = s2p.tile([128, 512], FP32, tag="ps_v", name="ps_v")
                        for cc in range(CC):
                            nc.tensor.matmul(ps,
                                             lhsT=y1T[:, cc, t * 128:(t + 1) * 128],
                                             rhs=wvT_s[:, cc, n2 * 512:(n2 + 1) * 512],
                                             start=(cc == 0), stop=(cc == CC - 1))
                        nc.scalar.copy(v_s[:, t, n2 * 8:(n2 + 1) * 8, 0:64],
                                       ps.rearrange("p (h d) -> p h d", h=8))

            # ===== Phase 3+4 fused: per head-pair QK projection + attention ===
            # prefetch proj weights (needed from phase 5)
            wpT_s = p_wp.tile([128, CC, C], BF16)
            nc.sync.dma_start(wpT_s, wpT[:].rearrange("(o p) m -> p o m", p=128))
            with tc.tile_pool(name="qk", bufs=2) as p_qk, \
                 tc.tile_pool(name="wqk", bufs=2) as p_wqk, \
                 tc.tile_pool(name="sexp", bufs=2) as p_se, \
                 tc.tile_pool(name="srow", bufs=2) as p_sr, \
                 tc.tile_pool(name="ps_qk", bufs=2, space="PSUM") as pp_qk, \
                 tc.tile_pool(name="ps_s", bufs=2, space="PSUM") as pp_s, \
                 tc.tile_pool(name="ps_o", bufs=1, space="PSUM") as pp_o:

                def emit_av(sexp, h, b, mc, po):
                    """AV with ones-column -> out^T [65, N]; row 64 = denom;
                    normalize and write straight into SBUF attnT."""
                    t0_ = b * NP
                    pso = pp_o.tile([128, 640], FP32, tag="ps_o", name="ps_o")
                    for j in range(MT):
                        mw = 128 if j < MT - 1 else N - 512
                        vj = v_s[:mw, MT * b + j, h, :]
                        nc.tensor.matmul(pso[0:65, 0:512], lhsT=vj,
                                         rhs=sexp[:mw, j, 0:512],
                                         start=(j == 0), stop=(j == MT - 1))
                        nc.tensor.matmul(pso[0:65, 512:N], lhsT=vj,
                                         rhs=sexp[:mw, j, 512:N],
                                         start=(j == 0), stop=(j == MT - 1))
                    # reciprocal of denominator row; replicate across
                    # partitions on GPSIMD (POOL engine is idle)
                    rrow = p_sr.tile([1, 608], FP32, tag="rrow", name="rrow")
                    nc.vector.reciprocal(rrow[:, 0:N], pso[64:65, 0:N])
                    rep = p_sr.tile([64, 608], FP32, tag="rep", name="rep")
                    nc.gpsimd.partition_broadcast(rep[:, 0:N], rrow[:, 0:N])
                    nc.vector.tensor_tensor(attnT[po:po + 64, mc, t0_:t0_ + N],
                                            pso[0:64, 0:N],
                                            rep[:, 0:N], op=ALU.mult)

                # compact token space for Q/K: u = 577*b + i <-> padded
                # column 640*b + i
                NREALA = BPC * N
                QCH = list(range(0, NREALA, 512)) + [NREALA]

                def qsegs(u0, u1):
                    segs = []
                    while u0 < u1:
                        b, i = u0 // N, u0 % N
                        take = min(u1 - u0, N - i)
                        segs.append((b * NP + i, take))
                        u0 += take
                    return segs

                pend = None
                for mc in range(CC):
                    # compute Q chunk (heads 2mc, 2mc+1) and K chunk on demand
                    wq_s = p_wqk.tile([128, CC, 128], BF16, tag="wq", name="wq_s")
                    nc.sync.dma_start(wq_s, wqkT_r[:, :, mc * 128:(mc + 1) * 128])
                    wk_s = p_wqk.tile([128, CC, 128], BF16, tag="wk", name="wk_s")
                    nc.sync.dma_start(
                        wk_s, wqkT_r[:, :, C + mc * 128:C + (mc + 1) * 128])
                    qc = p_qk.tile([128, T], BF16, tag="qc", name="qc")
                    kc = p_qk.tile([128, T], BF16, tag="kc", name="kc")
                    for dst, w_s in ((qc, wq_s), (kc, wk_s)):
                        for n5 in range(len(QCH) - 1):
                            u0, u1 = QCH[n5], QCH[n5 + 1]
                            cw = u1 - u0
                            ps = pp_qk.tile([128, 512], FP32, tag="ps_qk",
                                            name="ps_qk")
                            off = 0
                            for pc, ln in qsegs(u0, u1):
                                for cc in range(CC):
                                    nc.tensor.matmul(
                                        ps[:, off:off + ln], lhsT=w_s[:, cc, :],
                                        rhs=y1T[:, cc, pc:pc + ln],
                                        start=(cc == 0), stop=(cc == CC - 1))
                                off += ln
                            nc.vector.tensor_copy(dst[:, u0:u0 + cw],
                                                  ps[:, 0:cw])
                    for h in (2 * mc, 2 * mc + 1):
                        po = (h % 2) * 64
                        for b in range(BPC):
                            t0_ = b * NP
                            tq_ = b * N
                            QT = qc[po:po + 64, tq_:tq_ + N]
                            sexp = p_se.tile([128, MT, 640], BF16, tag="sexp",
                                             name="sexp")
                            for j in range(MT):
                                mw = 128 if j < MT - 1 else N - 512
                                KTj = kc[po:po + 64,
                                         tq_ + j * 128: tq_ + j * 128 + mw]
                                ps = pp_s.tile([128, 640], FP32, tag="ps_s",
                                               name="ps_s")
                                nc.tensor.matmul(ps[:mw, 0:512], lhsT=KTj,
                                                 rhs=QT[:, 0:512], start=True,
                                                 stop=True)
                                nc.tensor.matmul(ps[:mw, 512:N], lhsT=KTj,
                                                 rhs=QT[:, 512:N], start=True,
                                                 stop=True)
                                nc.scalar.activation(sexp[:mw, j, 0:N],
                                                     ps[:mw, 0:N],
                                                     AF.Exp, scale=SCALE)
                            # software pipeline: emit previous (h,b)'s AV now,
                            # giving its exps a full S-block of slack
                            if pend is not None:
                                emit_av(*pend)
                            pend = (sexp, h, b, mc, po)
                emit_av(*pend)

            # ===== Phase 5: proj + residual + LN2 + transpose =====
            for k in range(2):
                nc.sync.dma_start(wf1a[:, :, k * 512:(k + 1) * 512],
                                  wf1T_r[:, :, k * 512:(k + 1) * 512])
            with tc.tile_pool(name="s5", bufs=3) as s5, \
                 tc.tile_pool(name="s5p", bufs=4, space="PSUM") as s5p, \
                 tc.tile_pool(name="s5pt", bufs=2, space="PSUM") as s5pt:
                BP = load_row("c", bpj)
                for t in range(TT):
                    xr = s5.tile([128, C], FP32, tag="xr", name="xr")
                    nc.gpsimd.dma_start(xr, xp_r[t])
                    x2_t = s5.tile([128, C], FP32, tag="x2", name="x2_t")
                    for n2 in range(2):
                        ps = s5p.tile([128, 512], FP32, tag="ps_p", name="ps_p")
                        for cc in range(CC):
                            nc.tensor.matmul(
                                ps, lhsT=attnT[:, cc, t * 128:(t + 1) * 128],
                                             rhs=wpT_s[:, cc, n2 * 512:(n2 + 1) * 512],
                                             start=(cc == 0), stop=(cc == CC - 1))
                        sl = slice(n2 * 512, (n2 + 1) * 512)
                        nc.vector.tensor_tensor(x2_t[:, sl], ps, xr[:, sl],
                                                op=ALU.add)
                        nc.gpsimd.tensor_tensor(x2_t[:, sl], x2_t[:, sl],
                                                BP[:, sl], op=ALU.add)
                    nc.sync.dma_start(x2d_r[t], x2_t)
                    y2 = layernorm(x2_t, s5)
                    y2Ts = s5.tile([128, CC, 128], BF16, tag="y2Ts", name="y2Ts")
                    transpose_tile(y2, y2Ts, s5pt, "pst2", g2c, b2c)
                    nc.sync.dma_start(y2Td_r[:, :, t * 128:(t + 1) * 128],
                                      y2Ts)

        # ================= Phase 6: MLP (compact token space) ==========
        # the MLP runs over the 2308 real tokens only; compact index
        # u = 577*b + i maps to padded column 640*b + i.
        NREAL = BPC * N
        CHN = list(range(0, NREAL, CH)) + [NREAL]

        def compact_segments(u0, u1):
            segs = []
            while u0 < u1:
                b, i = u0 // N, u0 % N
                take = min(u1 - u0, N - i)
                segs.append((b * NP + i, take))
                u0 += take
            return segs

        xp_f = x2d[:]
        out_f = out[:]
        BF2 = load_row("c", bf2)
        with tc.tile_pool(name="wmlp", bufs=1) as p_wm, \
             tc.tile_pool(name="hT", bufs=1) as p_hT, \
             tc.tile_pool(name="s6", bufs=2) as s6, \
             tc.tile_pool(name="s6p1", bufs=4, space="PSUM") as s6p1, \
             tc.tile_pool(name="s6p2", bufs=4, space="PSUM") as s6p2:
            # first activation chunk ahead of the weight chunks on ACT
            y2c0 = s6.tile([128, CC, CH], BF16, tag="y2c", name="y2c")
            off0 = 0
            for pc, ln in compact_segments(CHN[0], CHN[1]):
                nc.scalar.dma_start(y2c0[:, :, off0:off0 + ln],
                                    y2Td_r[:, :, pc:pc + ln])
                off0 += ln
            # chunked weight loads on both HWDGE queues (SP + ACT) so the
            # first fc1 matmuls start ~3us after phase 5 instead of ~50us
            wf1T_s = p_wm.tile([128, CC, HID - 1024], BF16)
            for k in range(6):
                nc.scalar.dma_start(wf1T_s[:, :, k * 512:(k + 1) * 512],
                                    wf1T_r[:, :, 1024 + k * 512:1024 + (k + 1) * 512])
            wf2T_s = p_wm.tile([128, HC, C], BF16)
            wf2T_r = wf2T[:].rearrange("(o p) m -> p o m", p=128)
            for k in range(4):
                nc.sync.dma_start(wf2T_s[:, :, k * 256:(k + 1) * 256],
                                  wf2T_r[:, :, k * 256:(k + 1) * 256])
            for u in range(len(CHN) - 1):
                u0, u1 = CHN[u], CHN[u + 1]
                cw = u1 - u0
                if u == 0:
                    y2c = y2c0
                else:
                    y2c = s6.tile([128, CC, CH], BF16, tag="y2c", name="y2c")
                    off = 0
                    for pc, ln in compact_segments(u0, u1):
                        nc.gpsimd.dma_start(y2c[:, :, off:off + ln],
                                            y2Td_r[:, :, pc:pc + ln])
                        off += ln
                hT = p_hT.tile([128, HC, CH], BF16, tag="hT", name="hT")
                for hc in range(HC):
                    psf = s6p1.tile([128, CH], FP32, tag="ps_f1", name="ps_f1")
                    if hc < 8:
                        wsrc = wf1a[:, :, hc * 128:(hc + 1) * 128]
                    else:
                        wsrc = wf1T_s[:, :, (hc - 8) * 128:(hc - 7) * 128]
                    for cc in range(CC):
                        nc.tensor.matmul(psf[:, :cw], lhsT=wsrc[:, cc],
                                         rhs=y2c[:, cc, :cw],
                                         start=(cc == 0), stop=(cc == CC - 1))
                    nc.scalar.activation(hT[:, hc, :cw], psf[:, :cw], AF.Gelu,
                                         bias=bf1_s[:, hc:hc + 1])
                for tt_ in range((cw + 127) // 128):
                    m0 = tt_ * 128
                    mw2 = min(128, cw - m0)
                    segs = compact_segments(u0 + m0, u0 + m0 + mw2)
                    xr2 = s6.tile([128, C], FP32, tag="xr2", name="xr2")
                    soff = 0
                    for pc, ln in segs:
                        nc.gpsimd.dma_start(xr2[soff:soff + ln, :],
                                            xp_f[pc:pc + ln, :])
                        soff += ln
                    out_t = s6.tile([128, C], FP32, tag="out", name="out_t")
                    for n2 in range(2):
                        ps2 = s6p2.tile([128, 512], FP32, tag="ps_f2", name="ps_f2")
                        for hc in range(HC):
                            nc.tensor.matmul(
                                ps2[:mw2], lhsT=hT[:, hc, m0:m0 + mw2],
                                rhs=wf2T_s[:, hc, n2 * 512:(n2 + 1) * 512],
                                start=(hc == 0), stop=(hc == HC - 1))
                        sl = slice(n2 * 512, (n2 + 1) * 512)
                        nc.vector.tensor_tensor(out_t[:mw2, sl], ps2[:mw2],
                                                BF2[:mw2, sl], op=ALU.add)
                        nc.vector.tensor_tensor(out_t[:mw2, sl], out_t[:mw2, sl],
                                                xr2[:mw2, sl], op=ALU.add)
                    soff = 0
                    for pc, ln in segs:
                        nc.sync.dma_start(out_f[pc:pc + ln, :],
                                          out_t[soff:soff + ln, :])
                        soff += ln


def _build(nc, reps=1):
    io = _declare_io(nc)
    with tile.TileContext(nc) as tc:
        for _rep in range(reps):
            _build_once(nc, tc, io)
    return nc


_NC_CACHE = {}


def _get_nc(reps=1):
    if reps not in _NC_CACHE:
        nc = bacc.Bacc(None, target_bir_lowering=False)
        _build(nc, reps=reps)
        nc.compile()
        _NC_CACHE[reps] = nc
    return _NC_CACHE[reps]


def kernel(x, ln1_g, ln1_b, w_qkv, w_proj, b_proj, ln2_g, ln2_b,
           w_fc1, b_fc1, w_fc2, b_fc2, _trace=False, _trace_kwargs=None):
    nc = _get_nc()

    def bf(a):
        return np.ascontiguousarray(np.asarray(a, np.float32).T).astype(BF16NP)

    x = np.asarray(x, np.float32)
    shared = {
        "wqkT": bf(w_qkv[:2 * C]),
        "wvT": bf(w_qkv[2 * C:]),
        "wpT": bf(w_proj),
        "wf1T": bf(w_fc1),
        "wf2T": bf(w_fc2),
        "g1": np.asarray(ln1_g, np.float32),
        "b1": np.asarray(ln1_b, np.float32),
        "g2": np.asarray(ln2_g, np.float32),
        "b2": np.asarray(ln2_b, np.float32),
        "bpj": np.asarray(b_proj, np.float32),
        "bf1": np.asarray(b_fc1, np.float32),
        "bf2": np.asarray(b_fc2, np.float32),
    }
    xs = x.reshape(NCORES, BPC, N, C)
    xpad = np.zeros((NCORES, BPC, NP, C), np.float32)
    xpad[:, :, :N] = xs
    in_maps = [dict(shared, xp=np.ascontiguousarray(xpad[c].reshape(T, C)))
               for c in range(NCORES)]

    kw = {}
    if _trace:
        kw = dict(trace=True, trace_kwargs=_trace_kwargs or {})
    res = bass_utils.run_bass_kernel_spmd(nc, in_maps, core_ids=list(range(NCORES)),
                                          **kw)
    kernel.last_results = res
    outs = []
    for c in range(NCORES):
        oc = np.asarray(res.results[c]["out"]).reshape(BPC, NP, C)[:, :N]
        outs.append(oc)
    return np.concatenate(outs, axis=0).astype(np.float32)
